# revision 39
# baseline (speedup 1.0000x reference)
"""Trainium2 Bass kernel for a 16-head causal MHA layer.

Problem: x:[2,2048,1024] f32, wq/wk/wv/wo:[1024,1024] f32 (Linear-style
[out,in] weights), causal softmax attention with 16 heads of dim 64.

Sharding across the 8 NeuronCores: 2-way data parallel over batch x
4-way tensor parallel over heads.  Core c handles batch c//4 and the 4
heads 4*(c%4) .. 4*(c%4)+3 (feature slice of 256 rows of wq/wk/wv and
256 columns of wo).  Each core produces a partial [2048,1024] output
(its 4 heads' contribution, already projected through its wo slice);
the host sums the 4 partials per batch.

Device dataflow (all matmul inputs fp16, fp32 PSUM accumulation):
  - host uploads x already transposed per batch: xT [1024, 2048] fp16
  - qT/kT = W @ xT in [feat, token] layout; v in [token, feat] layout,
    with a constant-1 column appended per head (v|1)
  - scoresT[k,q] = kT_h.T-block @ qT_h (64-dim contraction), exp on ACT
    straight out of PSUM (no max subtraction: |scores/8| < ~4 so exp is
    safe in fp32/fp16), causal mask applied only on diagonal blocks via
    a precomputed 0/1 mask multiply
  - out_unnorm.T | l = (v|1).T-block @ expT accumulated over k blocks
    (the appended ones-column yields the softmax denominator l for free)
  - 1/l via a DRAM-roundtrip transpose to [128,x] + DVE reciprocal,
    broadcast back across partitions, multiply into out_unnorm.T
  - y = outT.T @ woT accumulated over the 256-dim feature slice
"""

import numpy as np

S = 2048          # sequence length (one batch per core)
D = 1024          # model dim
HL = 4            # heads handled per core
DH = 64           # head dim
F = HL * DH       # 256 local features
DC = D // 128     # 8 d_model chunks of 128
FC = F // 128     # 2 feature chunks of 128
NT = S // 128     # 16 token tiles
NQ = S // 512     # 4 query chunks of 512

_CACHE = {}


def _build_program(dbg=False):
    key = ("nc", dbg)
    if key in _CACHE:
        return _CACHE[key]

    import concourse.bacc as bacc
    import concourse.bass as bass
    import concourse.mybir as mybir
    import concourse.tile as tile

    f16 = mybir.dt.float16
    f32 = mybir.dt.float32
    Exp = mybir.ActivationFunctionType.Exp

    nc = bacc.Bacc("TRN2", target_bir_lowering=False, debug=False)

    xT_d = nc.dram_tensor("xT", [DC, 128, S], f16, kind="ExternalInput")
    wqT_d = nc.dram_tensor("wqT", [DC, 128, F], f16, kind="ExternalInput")
    wkT_d = nc.dram_tensor("wkT", [DC, 128, F], f16, kind="ExternalInput")
    wvT_d = nc.dram_tensor("wvT", [DC, 128, F], f16, kind="ExternalInput")
    woT_d = nc.dram_tensor("woT", [FC, 128, D], f16, kind="ExternalInput")
    mask_d = nc.dram_tensor("mask", [128, 896], f16, kind="ExternalInput")
    ident_d = nc.dram_tensor("ident", [128, 128], f16, kind="ExternalInput")
    y_d = nc.dram_tensor("y", [S, D], f16, kind="ExternalOutput")
    if dbg:
        qT_dbg = nc.dram_tensor("qT_dbg", [128, FC, S], f16, kind="ExternalOutput")
        kT_dbg = nc.dram_tensor("kT_dbg", [128, FC, S], f16, kind="ExternalOutput")
        v_dbg = nc.dram_tensor("v_dbg", [128, NT, HL, DH + 1], f16, kind="ExternalOutput")
        outT_dbg = nc.dram_tensor("outT_dbg", [128, FC, S], f16, kind="ExternalOutput")
        l_dbg = nc.dram_tensor("l_dbg", [HL * S], f32, kind="ExternalOutput")
        lt_dbg = nc.dram_tensor("lt_dbg", [128, HL * NT], f32, kind="ExternalOutput")

    with tile.TileContext(nc) as tc:
        with tc.tile_pool(name="const", bufs=1) as cpool, \
             tc.tile_pool(name="dscr", bufs=1,
                          space=bass.MemorySpace.DRAM) as dpool:
            l_dram = dpool.tile([HL * S], f32)
            xT = cpool.tile([128, DC, S], f16)
            wq = cpool.tile([128, DC, F], f16)
            wk = cpool.tile([128, DC, F], f16)
            wv = cpool.tile([128, DC, F], f16)
            wo = cpool.tile([128, FC, D], f16)
            mask = cpool.tile([128, 896], f16)
            ident = cpool.tile([128, 128], f16)
            qT = cpool.tile([128, FC, S], f16)
            kT = cpool.tile([128, FC, S], f16)
            v = cpool.tile([128, NT, HL, DH + 1], f16)
            outT = cpool.tile([128, FC, S], f16)
            l_row = cpool.tile([1, HL * S], f32)
            lT = cpool.tile([128, HL * NT], f32)
            recipT = cpool.tile([128, HL * NT], f32)
            recipT16 = cpool.tile([128, HL * NT], f16)

            # loads: spread issue across four engine queues so the wire
            # fills in parallel; weights arrive as single rearranged DMAs
            for dc in range(0, DC, 2):
                nc.sync.dma_start(xT[:, dc, :], xT_d[dc])
                nc.scalar.dma_start(xT[:, dc + 1, :], xT_d[dc + 1])
            for w_sb, w_d in ((wq, wqT_d), (wk, wkT_d), (wv, wvT_d)):
                nc.gpsimd.dma_start(
                    w_sb[:], w_d.rearrange("c p f -> p c f"))
            nc.gpsimd.dma_start(wo[:], woT_d.rearrange("c p f -> p c f"))
            nc.scalar.dma_start(mask[:], mask_d[:])
            nc.scalar.dma_start(ident[:], ident_d[:])

            # ones columns for the softmax-denominator trick
            nc.gpsimd.memset(v[:], 1.0)

            # ---- attention + normalize + output projection -------------
            # qc-major: all heads for query-chunk qc, then (lagged by one
            # chunk so every dependency is long ready) the softmax
            # normalization and wo projection for chunk qc-1.  The wo/bc
            # matmuls fill the PE bubbles of the exp-bound attention loop.
            with tc.tile_pool(name="sc_ps", bufs=2,
                              space=bass.MemorySpace.PSUM) as scp, \
                 tc.tile_pool(name="av_ps", bufs=2,
                              space=bass.MemorySpace.PSUM) as avp, \
                 tc.tile_pool(name="ybc_ps", bufs=2,
                              space=bass.MemorySpace.PSUM) as ybcp, \
                 tc.tile_pool(name="p_sb", bufs=4) as ppool, \
                 tc.tile_pool(name="y_sb", bufs=4) as ysb_pool:

                def proj_qk(t5):
                    # one 512-token slice of the q/k projections (all fc)
                    for w_sb, dstT in ((wq, qT), (wk, kT)):
                        for fc in range(FC):
                            ps = ybcp.tile([128, 512], f32, tag="ybc",
                                           name=f"ps_{t5}_{fc}")
                            for dc in range(DC):
                                nc.tensor.matmul(
                                    ps[:],
                                    w_sb[:, dc, fc * 128:(fc + 1) * 128],
                                    xT[:, dc, t5 * 512:(t5 + 1) * 512],
                                    start=(dc == 0), stop=(dc == DC - 1))
                            nc.vector.tensor_copy(
                                dstT[:, fc, t5 * 512:(t5 + 1) * 512], ps[:])

                def proj_v(tts):
                    for tt in tts:
                        psv = ybcp.tile([128, F], f32, tag="ybc",
                                        name=f"psv_{tt}")
                        for dc in range(DC):
                            nc.tensor.matmul(
                                psv[:],
                                xT[:, dc, tt * 128:(tt + 1) * 128],
                                wv[:, dc, :],
                                start=(dc == 0), stop=(dc == DC - 1))
                        nc.vector.tensor_copy(
                            v[:, tt, :, 0:DH],
                            psv.rearrange("p (h d) -> p h d", h=HL))

                def att_hc(qc, hc):
                    if True:
                        avs = []
                        for hp2 in range(2):
                            av = avp.tile([DH + 1, 512], f32, tag="av",
                                          name=f"av_{hc}_{qc}_{hp2}")
                            avs.append(av)
                        for g in range(qc + 1):
                            diag = (g == qc)
                            for half in range(2):
                                # (offset, width) of each k-block's valid
                                # q-span inside the p tile; diagonal blocks
                                # are clipped to q >= k_block_start
                                if diag:
                                    rs = [2 * half, 2 * half + 1]
                                    spans = [(128 * r, 512 - 128 * r)
                                             for r in rs]
                                else:
                                    spans = [(0, 512), (0, 512)]
                                offs = [0, spans[0][1]]
                                scs = []
                                for hp2 in range(2):
                                    sc = scp.tile([128, 1024], f32, tag="sc",
                                                  name=f"sc_{hc}_{qc}_{g}_{half}_{hp2}")
                                    scs.append(sc)
                                for r2 in range(2):
                                    kb = 4 * g + 2 * half + r2
                                    qo, w = spans[r2]
                                    for hp2 in range(2):
                                        hp = hp2 * 64
                                        nc.tensor.matmul(
                                            scs[hp2][:, offs[r2]:offs[r2] + w],
                                            kT[hp:hp + 64, hc,
                                               kb * 128:(kb + 1) * 128],
                                            qT[hp:hp + 64, hc,
                                               qc * 512 + qo:(qc + 1) * 512],
                                            start=True, stop=True,
                                            tile_position=(hp, 0))
                                width = offs[1] + spans[1][1]
                                for hp2 in range(2):
                                    h = hc * 2 + hp2
                                    p_sb = ppool.tile([128, 1024], f16,
                                                      tag=f"p{hp2}",
                                                      name=f"p_{hc}_{qc}_{g}_{half}_{hp2}")
                                    nc.scalar.activation(
                                        p_sb[:, 0:width],
                                        scs[hp2][:, 0:width], Exp)
                                    if diag:
                                        # only the first 128 columns of a
                                        # clipped block straddle the diagonal
                                        for r2 in range(2):
                                            nc.vector.tensor_mul(
                                                p_sb[:, offs[r2]:offs[r2] + 128],
                                                p_sb[:, offs[r2]:offs[r2] + 128],
                                                mask[:, 384:512])
                                    for r2 in range(2):
                                        kb = 4 * g + 2 * half + r2
                                        qo, w = spans[r2]
                                        nc.tensor.matmul(
                                            avs[hp2][:, qo:512],
                                            v[:, kb, h, :],
                                            p_sb[:, offs[r2]:offs[r2] + w],
                                            start=(kb == 0),
                                            stop=(kb == 4 * qc + 3))
                        for hp2 in range(2):
                            h = hc * 2 + hp2
                            hp = hp2 * 64
                            nc.vector.tensor_copy(
                                outT[hp:hp + 64, hc, qc * 512:(qc + 1) * 512],
                                avs[hp2][0:DH, :])
                            # denominators via Scalar so chain DMAs never
                            # stall the Vector queue
                            nc.scalar.copy(
                                l_row[0:1, h * S + qc * 512:
                                      h * S + (qc + 1) * 512],
                                avs[hp2][DH:DH + 1, :])
                            seg = slice(h * S + qc * 512,
                                        h * S + (qc + 1) * 512)
                            nc.sync.dma_start(l_dram[seg], l_row[0:1, seg])
                            nc.sync.dma_start(
                                lT[:, h * NT + 4 * qc:h * NT + 4 * qc + 4],
                                l_dram[seg].rearrange("(t p) -> p t", p=128))

                def norm_heads(qc, heads):
                    # 1/l on the [q-partition] transposed copy, broadcast
                    # back over the 64 dh rows with a K=128 matmul against
                    # the identity (no DMA in this chain)
                    for h in heads:
                        hc = h // 2
                        hp = (h % 2) * 64
                        cs = slice(h * NT + 4 * qc, h * NT + 4 * qc + 4)
                        nc.vector.reciprocal(recipT[:, cs], lT[:, cs])
                        nc.vector.tensor_copy(recipT16[:, cs], recipT[:, cs])
                        bc = ybcp.tile([64, 512], f32, tag="ybc",
                                       name=f"bc_{h}_{qc}")
                        for t4 in range(4):
                            col = h * NT + 4 * qc + t4
                            nc.tensor.matmul(
                                bc[:, t4 * 128:(t4 + 1) * 128],
                                recipT16[:, col:col + 1]
                                .to_broadcast((128, DH)),
                                ident[:],
                                start=True, stop=True)
                        nc.vector.tensor_mul(
                            outT[hp:hp + 64, hc, qc * 512:(qc + 1) * 512],
                            outT[hp:hp + 64, hc, qc * 512:(qc + 1) * 512],
                            bc[:])

                def wo_chunk(qc):
                    for qt in range(4 * qc, 4 * qc + 4):
                        for oc in range(2):
                            yps = ybcp.tile([128, 512], f32, tag="ybc",
                                            name=f"yps_{qt}_{oc}")
                            for fc in range(FC):
                                nc.tensor.matmul(
                                    yps[:],
                                    outT[:, fc, qt * 128:(qt + 1) * 128],
                                    wo[:, fc, oc * 512:(oc + 1) * 512],
                                    start=(fc == 0), stop=(fc == FC - 1))
                            ysb = ysb_pool.tile([128, 512], f16, tag="ysb",
                                                name=f"ysb_{qt}_{oc}")
                            nc.vector.tensor_copy(ysb[:], yps[:])
                            nc.sync.dma_start(
                                y_d[qt * 128:(qt + 1) * 128,
                                    oc * 512:(oc + 1) * 512],
                                ysb[:])

                proj_qk(0)
                proj_v(range(0, 4))
                for qc in range(NQ - 1):
                    att_hc(qc, 0)
                    att_hc(qc, 1)
                    proj_qk(qc + 1)
                    proj_v(range(4 * (qc + 1), 4 * (qc + 2)))
                    if qc >= 1:
                        norm_heads(qc - 1, range(HL))
                        wo_chunk(qc - 1)
                att_hc(NQ - 1, 0)
                norm_heads(NQ - 2, range(HL))
                wo_chunk(NQ - 2)
                att_hc(NQ - 1, 1)
                norm_heads(NQ - 1, [0, 1])
                norm_heads(NQ - 1, [2, 3])
                wo_chunk(NQ - 1)

            if dbg:
                nc.sync.dma_start(qT_dbg[:], qT[:])
                nc.sync.dma_start(kT_dbg[:], kT[:])
                nc.sync.dma_start(v_dbg[:], v[:])
                nc.sync.dma_start(outT_dbg[:], outT[:])
                nc.sync.dma_start(l_dbg[:], l_row[0:1, :])
                nc.sync.dma_start(lt_dbg[:], lT[:])

    nc.compile()

    from concourse.bass_interp import get_hw_module
    nc.m = get_hw_module(nc.m)

    _CACHE[key] = nc
    return nc


def _make_mask():
    # mask[p, j] = 1 where (j - p) >= 384; slices of width 512 at offset
    # 384-128*r give the causal mask for a diagonal block at relative
    # position r (k block kb = 4*qc + r vs the 512-wide q chunk qc)
    j = np.arange(896)[None, :]
    p = np.arange(128)[:, None]
    return ((j - p) >= 384).astype(np.float16)


def kernel(x, wq, wk, wv, wo):
    x = np.asarray(x, dtype=np.float32)
    wq = np.asarray(wq, dtype=np.float32)
    wk = np.asarray(wk, dtype=np.float32)
    wv = np.asarray(wv, dtype=np.float32)
    wo = np.asarray(wo, dtype=np.float32)

    from concourse import bass_utils

    nc = _build_program()
    mask = _make_mask()

    in_maps = []
    for c in range(8):
        b = c // 4
        hg = c % 4
        fs = slice(hg * F, (hg + 1) * F)
        xT = np.ascontiguousarray(x[b].T).astype(np.float16).reshape(DC, 128, S)
        wqT = np.ascontiguousarray((wq[fs, :] * 0.125).T).astype(np.float16)
        wkT = np.ascontiguousarray(wk[fs, :].T).astype(np.float16)
        wvT = np.ascontiguousarray(wv[fs, :].T).astype(np.float16)
        woT = np.ascontiguousarray(wo[:, fs].T).astype(np.float16)
        in_maps.append({
            "xT": xT,
            "wqT": wqT.reshape(DC, 128, F),
            "wkT": wkT.reshape(DC, 128, F),
            "wvT": wvT.reshape(DC, 128, F),
            "woT": woT.reshape(FC, 128, D),
            "mask": mask,
            "ident": np.eye(128, dtype=np.float16),
        })

    res = bass_utils.run_bass_kernel_spmd(nc, in_maps, core_ids=list(range(8)))
    ys = [res.results[c]["y"].astype(np.float32) for c in range(8)]
    out = np.stack([ys[0] + ys[1] + ys[2] + ys[3],
                    ys[4] + ys[5] + ys[6] + ys[7]])
    return out
